# revision 6
# baseline (speedup 1.0000x reference)
"""Distributed Trainium2 kernel for a 16-head self-attention block.

Model (reference):
    qkv = x @ W_qkv + b_qkv ; q,k,v per 16 heads (head_dim 64)
    attn = softmax(q k^T / 8) ; out = (attn @ v heads concat) @ W_out + b_out
Shapes: x [2, 2048, 1024], W_qkv [1024, 3072], W_out [1024, 1024].

Sharding (8 NeuronCores): 2 batch groups x 4 cores; each core owns one batch
element and 4 of the 16 heads (Megatron-style column-parallel QKV + row-
parallel out-proj). Each core computes its partial out-projection
[2048, 1024]; the host sums the 4 partials per batch (the Megatron
all-reduce, performed at unshard time) and adds the output bias.

Numerical notes:
  * softmax runs without max-subtraction: scores/8 here are bounded ~|2.5|.
  * the V bias folds out of attention exactly (softmax rows sum to 1), so
    bv@W_out is added to the host-side output bias instead of on device.

Device dataflow per core (no transposes anywhere):
  x^T [1024, 2048] (host pre-transposed) ->
  Q^T,K^T [256, 2048] = Wq^T x^T (+bias on eviction);  V [2048, 4, 65]
    (per-head 64 dims + a ones column, which makes the PV matmul emit the
    softmax denominator as row 64 of U^T)
  per head h, per 1024-query half: for each 128-key tile:
    scores^T [128 k, 1024 q] in a 2-bank PSUM tile (2 matmuls, shared
    stationary), exp via one 1024-wide ScalarE activation -> E [128, 1024]
    U^T [65, 512] += V[kt]^T E  (V stationary 65 cols, E moving 512)
  normalization: rc[1,512] = 1/denom row (DVE), GpSimd partition-broadcast
    to [64,512], one DVE multiply evicts U^T/denom into ut (bf16).
  out partial [2048, 1024] = (U^T)^T @ Wo_rows, emitted per query half so
  it overlaps the other half's attention.
"""

import contextlib
import os

import numpy as np

import concourse.bacc as bacc
import concourse.mybir as mybir
import concourse.tile as tile
from concourse import bass_utils

F32 = mybir.dt.float32
AF = mybir.ActivationFunctionType

S = 2048          # sequence length (one batch element per core)
E = 1024          # embedding dim
HD = 64           # head dim
NH = 4            # heads per core
DQ = NH * HD      # per-core q/k/v width (256)
ET = E // 128     # embedding tiles (8)
ST = S // 128     # sequence tiles (16)
QB = S // 512     # 512-wide query blocks (4)

MODE = os.environ.get("ATTN_MM_DTYPE", "bf16")  # bf16 | f32r | f32

_CACHED = {}


def build_core_graph(mode=MODE):
    MD = {"f32r": mybir.dt.float32r, "f32": F32, "bf16": mybir.dt.bfloat16}[mode]

    nc = bacc.Bacc("TRN2", target_bir_lowering=False, debug=False, num_devices=8)

    xt_d = nc.dram_tensor("xt", [E, S], MD, kind="ExternalInput")
    wq_d = nc.dram_tensor("wq", [E, DQ], MD, kind="ExternalInput")
    wk_d = nc.dram_tensor("wk", [E, DQ], MD, kind="ExternalInput")
    wv_d = nc.dram_tensor("wv", [E, DQ], MD, kind="ExternalInput")
    bq_d = nc.dram_tensor("bq", [DQ, 1], F32, kind="ExternalInput")
    bk_d = nc.dram_tensor("bk", [DQ, 1], F32, kind="ExternalInput")
    wo_d = nc.dram_tensor("wo", [DQ, E], MD, kind="ExternalInput")
    out_d = nc.dram_tensor("out", [S, E], mybir.dt.bfloat16, kind="ExternalOutput")

    with tile.TileContext(nc) as tc:
        with contextlib.ExitStack() as ctx:
            # ---- persistent SBUF pools ------------------------------------
            pers = ctx.enter_context(tc.tile_pool(name="pers", bufs=1))

            def ptile(shape, dtype, nm):
                return pers.tile(shape, dtype, tag=nm, name=nm)

            qt = [ptile([128, S], MD, f"qt{t}") for t in range(2)]
            ones_c = ptile([128, NH], F32, "ones_c")
            kt_sb = [ptile([128, S], MD, f"kt{t}") for t in range(2)]
            v_sb = [ptile([128, NH, HD + 1], MD, f"v{st}") for st in range(ST)]
            ut = [ptile([128, S], MD, f"ut{t}") for t in range(2)]
            wo_sb = [ptile([128, E], MD, f"wo{t}") for t in range(2)]
            bq_sb = [ptile([128, 1], F32, f"bq{t}") for t in range(2)]
            bk_sb = [ptile([128, 1], F32, f"bk{t}") for t in range(2)]

            nc.vector.memset(ones_c[:], 1.0)
            for t in range(2):
                nc.sync.dma_start(wo_sb[t][:], wo_d[t * 128:(t + 1) * 128, :])
                nc.sync.dma_start(bq_sb[t][:], bq_d[t * 128:(t + 1) * 128, :])
                nc.sync.dma_start(bk_sb[t][:], bk_d[t * 128:(t + 1) * 128, :])

            # ---- stage A: QKV projections ---------------------------------
            # Order: K/Q for head-pair 0 first so attention can start while
            # the rest of stage A still runs.
            with tc.tile_pool(name="stage_a", bufs=1) as apool, \
                 tc.tile_pool(name="a_ps", bufs=3, space="PSUM") as a_ps, \
                 tc.tile_pool(name="warm_ps", bufs=1, space="PSUM") as warm_ps:
                def atile(shape, nm):
                    return apool.tile(shape, MD, tag=nm, name=nm)

                xts = [atile([128, S], f"xt{et}") for et in range(ET)]
                wqs = [atile([128, DQ], f"wq{et}") for et in range(ET)]
                wks = [atile([128, DQ], f"wk{et}") for et in range(ET)]
                wvs = [atile([128, DQ], f"wv{et}") for et in range(ET)]
                for et in range(ET):
                    sl = slice(et * 128, (et + 1) * 128)
                    nc.sync.dma_start(xts[et][:], xt_d[sl, :])
                    nc.sync.dma_start(wqs[et][:], wq_d[sl, :])
                    nc.sync.dma_start(wks[et][:], wk_d[sl, :])
                    nc.sync.dma_start(wvs[et][:], wv_d[sl, :])

                # HAM warm-up: keep the PE busy during the input-DMA window
                # so the clock-gate releases (K=8/8) before the real matmul
                # stream begins. Garbage data, never read.
                warm_sb = apool.tile([128, 512], MD, tag="warm", name="warm")
                nc.vector.memset(warm_sb[:], 0.0)
                wps = warm_ps.tile([128, 512], F32, tag="wps", name="wps")
                for _ in range(56):
                    nc.tensor.matmul(wps[:], warm_sb[:, 0:128], warm_sb[:],
                                     start=True, stop=True)

                # Q^T / K^T: [dq-tile 128, s-block 512] += w[e,dq].T @ x^T[e,s]
                def qk_proj(t):
                    tsl = slice(t * 128, (t + 1) * 128)
                    for qb in range(QB):
                        qsl = slice(qb * 512, (qb + 1) * 512)
                        pk = a_ps.tile([128, 512], F32, tag="aps", name="aps")
                        for et in range(ET):
                            nc.tensor.matmul(pk[:], wks[et][:, tsl], xts[et][:, qsl],
                                             start=(et == 0), stop=(et == ET - 1))
                        nc.vector.tensor_scalar_add(kt_sb[t][:, qsl], pk[:], bk_sb[t][:])
                        pq = a_ps.tile([128, 512], F32, tag="aps", name="aps")
                        for et in range(ET):
                            nc.tensor.matmul(pq[:], wqs[et][:, tsl], xts[et][:, qsl],
                                             start=(et == 0), stop=(et == ET - 1))
                        nc.vector.tensor_scalar_add(qt[t][:, qsl], pq[:], bq_sb[t][:])

                qk_proj(0)
                # V: [s-tile 128, dv 256] += x^T[e,s].T @ wv[e,dv]
                for st in range(ST):
                    ssl = slice(st * 128, (st + 1) * 128)
                    pv = a_ps.tile([128, DQ], F32, tag="aps", name="aps")
                    for et in range(ET):
                        nc.tensor.matmul(pv[:], xts[et][:, ssl], wvs[et][:],
                                         start=(et == 0), stop=(et == ET - 1))
                    for h in range(NH):
                        nc.vector.tensor_copy(v_sb[st][:, h, 0:HD],
                                              pv[:, h * HD:(h + 1) * HD])
                    nc.vector.tensor_copy(v_sb[st][:, :, HD:HD + 1],
                                          ones_c[:].rearrange("p (h d) -> p h d", h=NH))
                qk_proj(1)

            # ---- stage B: attention ---------------------------------------
            # PSUM budget (8 banks): sc 2 bufs x 2 banks = 4, pv (one
            # [65,1024] tile per (h,qh)) 2 bufs x 2 banks = 4.
            with tc.tile_pool(name="e_sb", bufs=4) as e_pool, \
                 tc.tile_pool(name="rc_sb", bufs=2) as rc_pool, \
                 tc.tile_pool(name="rcb_sb", bufs=2) as rcb_pool, \
                 tc.tile_pool(name="sc_ps", bufs=2, space="PSUM") as sc_ps, \
                 tc.tile_pool(name="pv_ps", bufs=2, space="PSUM") as pv_ps:
                for qh in range(2):  # 1024-wide query halves
                    for h in range(NH):
                        t, po = h // 2, (h % 2) * 64
                        psl = slice(po, po + 64)
                        pvp = pv_ps.tile([HD + 1, 1024], F32, tag="pv", name="pv")
                        for kt in range(ST):
                            ksl = slice(kt * 128, (kt + 1) * 128)
                            sc = sc_ps.tile([128, 1024], F32, tag="sc", name="sc")
                            for q2 in range(2):
                                qsl = slice(qh * 1024 + q2 * 512,
                                            qh * 1024 + (q2 + 1) * 512)
                                nc.tensor.matmul(sc[:, q2 * 512:(q2 + 1) * 512],
                                                 kt_sb[t][psl, ksl], qt[t][psl, qsl],
                                                 start=True, stop=True)
                            e_sb = e_pool.tile([128, 1024], MD, tag="e", name="e")
                            nc.scalar.activation(e_sb[:], sc[:], AF.Exp, scale=0.125)
                            for q2 in range(2):
                                nc.tensor.matmul(pvp[:, q2 * 512:(q2 + 1) * 512],
                                                 v_sb[kt][:, h, :],
                                                 e_sb[:, q2 * 512:(q2 + 1) * 512],
                                                 start=(kt == 0), stop=(kt == ST - 1))
                        # normalize: U^T[0:64,:] * broadcast(1/denom row)
                        ssl = slice(qh * 1024, (qh + 1) * 1024)
                        rc = rc_pool.tile([1, 1024], F32, tag="rc", name="rc")
                        nc.vector.reciprocal(rc[:], pvp[HD:HD + 1, :])
                        rcb = rcb_pool.tile([HD, 1024], F32, tag="rcb", name="rcb")
                        nc.gpsimd.partition_broadcast(rcb[:], rc[:], channels=HD)
                        nc.vector.tensor_tensor(ut[t][psl, ssl], pvp[0:HD, :],
                                                rcb[:], mybir.AluOpType.mult)

            # ---- stage C: out-projection ----------------------------------
            with tc.tile_pool(name="o_sb", bufs=3) as o_pool, \
                 tc.tile_pool(name="op_ps", bufs=2, space="PSUM") as op_ps:
                for st in range(ST):
                    ssl = slice(st * 128, (st + 1) * 128)
                    o_sb = o_pool.tile([128, E], mybir.dt.bfloat16, tag="o", name="o")
                    for ob in range(2):
                        osl = slice(ob * 512, (ob + 1) * 512)
                        op = op_ps.tile([128, 512], F32, tag="op", name="op")
                        for t in range(2):
                            nc.tensor.matmul(op[:], ut[t][:, ssl],
                                             wo_sb[t][:, osl],
                                             start=(t == 0), stop=(t == 1))
                        nc.vector.tensor_copy(o_sb[:, osl], op[:])
                    nc.sync.dma_start(out_d[ssl, :], o_sb[:])

    nc.compile()
    return nc


def _get_graph():
    if "nc" not in _CACHED:
        _CACHED["nc"] = build_core_graph()
    return _CACHED["nc"]


def _np_mode_dtype():
    if MODE == "bf16":
        import ml_dtypes
        return ml_dtypes.bfloat16
    return np.float32


def kernel(x, W_qkv, b_qkv, W_out, b_out):
    x = np.asarray(x, dtype=np.float32)
    W_qkv = np.asarray(W_qkv, dtype=np.float32)
    b_qkv = np.asarray(b_qkv, dtype=np.float32)
    W_out = np.asarray(W_out, dtype=np.float32)
    b_out = np.asarray(b_out, dtype=np.float32)

    nc = _get_graph()
    md = _np_mode_dtype()

    Wq, Wk, Wv = W_qkv[:, 0:E], W_qkv[:, E:2 * E], W_qkv[:, 2 * E:3 * E]
    bq, bk, bv = b_qkv[0:E], b_qkv[E:2 * E], b_qkv[2 * E:3 * E]

    in_maps = []
    for c in range(8):
        b, hg = c // 4, c % 4
        cols = slice(DQ * hg, DQ * hg + DQ)
        in_maps.append({
            "xt": np.ascontiguousarray(x[b].T).astype(md),
            "wq": np.ascontiguousarray(Wq[:, cols]).astype(md),
            "wk": np.ascontiguousarray(Wk[:, cols]).astype(md),
            "wv": np.ascontiguousarray(Wv[:, cols]).astype(md),
            "bq": np.ascontiguousarray(bq[cols].reshape(DQ, 1)),
            "bk": np.ascontiguousarray(bk[cols].reshape(DQ, 1)),
            "wo": np.ascontiguousarray(W_out[cols, :]).astype(md),
        })

    res = bass_utils.run_bass_kernel_spmd(nc, in_maps, core_ids=list(range(8)))
    _CACHED["last_results"] = res

    b_eff = (b_out.astype(np.float64) +
             bv.astype(np.float64) @ W_out.astype(np.float64))
    out = np.empty((2, S, E), np.float32)
    for b in range(2):
        acc = np.zeros((S, E), np.float64)
        for hg in range(4):
            acc += res.results[4 * b + hg]["out"].astype(np.float64)
        out[b] = (acc + b_eff).astype(np.float32)
    return out


# revision 7
# speedup vs baseline: 1.0117x; 1.0117x over previous
"""Distributed Trainium2 kernel for a 16-head self-attention block.

Model (reference):
    qkv = x @ W_qkv + b_qkv ; q,k,v per 16 heads (head_dim 64)
    attn = softmax(q k^T / 8) ; out = (attn @ v heads concat) @ W_out + b_out
Shapes: x [2, 2048, 1024], W_qkv [1024, 3072], W_out [1024, 1024].

Sharding (8 NeuronCores): 2 batch groups x 4 cores; each core owns one batch
element and 4 of the 16 heads (Megatron-style column-parallel QKV + row-
parallel out-proj). Each core computes its partial out-projection
[2048, 1024]; the host sums the 4 partials per batch (the Megatron
all-reduce, performed at unshard time) and adds the output bias.

Numerical notes:
  * softmax runs without max-subtraction: scores/8 here are bounded ~|2.5|.
  * the V bias folds out of attention exactly (softmax rows sum to 1), so
    bv@W_out is added to the host-side output bias instead of on device.

Device dataflow per core (no transposes anywhere):
  x^T [1024, 2048] (host pre-transposed) ->
  Q^T,K^T [256, 2048] = Wq^T x^T (+bias on eviction);  V [2048, 4, 65]
    (per-head 64 dims + a ones column, which makes the PV matmul emit the
    softmax denominator as row 64 of U^T)
  per head h, per 1024-query half: for each 128-key tile:
    scores^T [128 k, 1024 q] in a 2-bank PSUM tile (2 matmuls, shared
    stationary), exp via one 1024-wide ScalarE activation -> E [128, 1024]
    U^T [65, 512] += V[kt]^T E  (V stationary 65 cols, E moving 512)
  normalization: rc[1,512] = 1/denom row (DVE), GpSimd partition-broadcast
    to [64,512], one DVE multiply evicts U^T/denom into ut (bf16).
  out partial [2048, 1024] = (U^T)^T @ Wo_rows, emitted per query half so
  it overlaps the other half's attention.
"""

import contextlib
import os

import numpy as np

import concourse.bacc as bacc
import concourse.mybir as mybir
import concourse.tile as tile
from concourse import bass_utils

F32 = mybir.dt.float32
AF = mybir.ActivationFunctionType

S = 2048          # sequence length (one batch element per core)
E = 1024          # embedding dim
HD = 64           # head dim
NH = 4            # heads per core
DQ = NH * HD      # per-core q/k/v width (256)
ET = E // 128     # embedding tiles (8)
ST = S // 128     # sequence tiles (16)
QB = S // 512     # 512-wide query blocks (4)

MODE = os.environ.get("ATTN_MM_DTYPE", "bf16")  # bf16 | f32r | f32

_CACHED = {}


def build_core_graph(mode=MODE):
    MD = {"f32r": mybir.dt.float32r, "f32": F32, "bf16": mybir.dt.bfloat16}[mode]

    nc = bacc.Bacc("TRN2", target_bir_lowering=False, debug=False, num_devices=8)

    xt_d = nc.dram_tensor("xt", [E, S], MD, kind="ExternalInput")
    wq_d = nc.dram_tensor("wq", [E, DQ], MD, kind="ExternalInput")
    wk_d = nc.dram_tensor("wk", [E, DQ], MD, kind="ExternalInput")
    wv_d = nc.dram_tensor("wv", [E, DQ], MD, kind="ExternalInput")
    bq_d = nc.dram_tensor("bq", [DQ, 1], F32, kind="ExternalInput")
    bk_d = nc.dram_tensor("bk", [DQ, 1], F32, kind="ExternalInput")
    wo_d = nc.dram_tensor("wo", [DQ, E], MD, kind="ExternalInput")
    out_d = nc.dram_tensor("out", [S, E], mybir.dt.bfloat16, kind="ExternalOutput")

    with tile.TileContext(nc) as tc:
        with contextlib.ExitStack() as ctx:
            # ---- persistent SBUF pools ------------------------------------
            pers = ctx.enter_context(tc.tile_pool(name="pers", bufs=1))

            def ptile(shape, dtype, nm):
                return pers.tile(shape, dtype, tag=nm, name=nm)

            qt = [ptile([128, S], MD, f"qt{t}") for t in range(2)]
            ones_c = ptile([128, NH], F32, "ones_c")
            kt_sb = [ptile([128, S], MD, f"kt{t}") for t in range(2)]
            v_sb = [ptile([128, NH, HD + 1], MD, f"v{st}") for st in range(ST)]
            ut = [ptile([128, S], MD, f"ut{t}") for t in range(2)]
            wo_sb = [ptile([128, E], MD, f"wo{t}") for t in range(2)]
            bq_sb = [ptile([128, 1], F32, f"bq{t}") for t in range(2)]
            bk_sb = [ptile([128, 1], F32, f"bk{t}") for t in range(2)]

            nc.vector.memset(ones_c[:], 1.0)
            for t in range(2):
                nc.sync.dma_start(wo_sb[t][:], wo_d[t * 128:(t + 1) * 128, :])
                nc.sync.dma_start(bq_sb[t][:], bq_d[t * 128:(t + 1) * 128, :])
                nc.sync.dma_start(bk_sb[t][:], bk_d[t * 128:(t + 1) * 128, :])

            # x^T and weight tiles live until the woven K1/Q1 block is done.
            apool = ctx.enter_context(tc.tile_pool(name="ab_sbuf", bufs=1))

            def atile(shape, nm):
                return apool.tile(shape, MD, tag=nm, name=nm)

            xts = [atile([128, S], f"xt{et}") for et in range(ET)]
            wqs = [atile([128, DQ], f"wq{et}") for et in range(ET)]
            wks = [atile([128, DQ], f"wk{et}") for et in range(ET)]
            wvs = [atile([128, DQ], f"wv{et}") for et in range(ET)]
            for et in range(ET):
                sl = slice(et * 128, (et + 1) * 128)
                nc.sync.dma_start(xts[et][:], xt_d[sl, :])
                nc.sync.dma_start(wqs[et][:], wq_d[sl, :])
                nc.sync.dma_start(wks[et][:], wk_d[sl, :])
                nc.sync.dma_start(wvs[et][:], wv_d[sl, :])

            # ---- stage A-pre: warmup + K/Q for head-pair 0 ----------------
            with tc.tile_pool(name="a_ps", bufs=4, space="PSUM") as a_ps:
                # HAM warm-up: keep the PE busy during the input-DMA window
                # so the clock-gate releases (K=8/8) before the real matmul
                # stream begins. Garbage data, never read.
                warm_sb = apool.tile([128, 512], MD, tag="warm", name="warm")
                nc.vector.memset(warm_sb[:], 0.0)
                wps = a_ps.tile([128, 512], F32, tag="wps", name="wps")
                for _ in range(56):
                    nc.tensor.matmul(wps[:], warm_sb[:, 0:128], warm_sb[:],
                                     start=True, stop=True)

                tsl = slice(0, 128)
                for qb in range(QB):
                    qsl = slice(qb * 512, (qb + 1) * 512)
                    pk = a_ps.tile([128, 512], F32, tag="aps", name="aps")
                    for et in range(ET):
                        nc.tensor.matmul(pk[:], wks[et][:, tsl], xts[et][:, qsl],
                                         start=(et == 0), stop=(et == ET - 1))
                    nc.vector.tensor_scalar_add(kt_sb[0][:, qsl], pk[:], bk_sb[0][:])
                    pq = a_ps.tile([128, 512], F32, tag="aps", name="aps")
                    for et in range(ET):
                        nc.tensor.matmul(pq[:], wqs[et][:, tsl], xts[et][:, qsl],
                                         start=(et == 0), stop=(et == ET - 1))
                    nc.vector.tensor_scalar_add(qt[0][:, qsl], pq[:], bq_sb[0][:])

            # ---- stage B: attention, two heads of a pair interleaved ------
            # PSUM budget (8 banks): sc ring 2 bufs x 2 banks = 4,
            # pvA + pvB [65,1024] 1 buf x 2 banks each = 4.
            # The V projection weaves into block 1's sc ring; K1/Q1 weave
            # into block 2's, so ScalarE starts exp'ing ~40us earlier.
            with tc.tile_pool(name="e_sb", bufs=4) as e_pool, \
                 tc.tile_pool(name="rc_sb", bufs=2) as rc_pool, \
                 tc.tile_pool(name="rcb_sb", bufs=2) as rcb_pool, \
                 tc.tile_pool(name="sc_ps", bufs=2, space="PSUM") as sc_ps, \
                 tc.tile_pool(name="pv_ps", bufs=1, space="PSUM") as pv_ps:

                def weave_v(kt):
                    ssl = slice(kt * 128, (kt + 1) * 128)
                    pv = sc_ps.tile([128, DQ], F32, tag="sc", name="sc")
                    for et in range(ET):
                        nc.tensor.matmul(pv[:], xts[et][:, ssl], wvs[et][:],
                                         start=(et == 0), stop=(et == ET - 1))
                    for hh in range(NH):
                        nc.vector.tensor_copy(v_sb[kt][:, hh, 0:HD],
                                              pv[:, hh * HD:(hh + 1) * HD])
                    nc.vector.tensor_copy(v_sb[kt][:, :, HD:HD + 1],
                                          ones_c[:].rearrange("p (h d) -> p h d", h=NH))

                def weave_kq1(kt):
                    if kt >= 8:
                        return
                    wsrc, dst, bias = ((wks, kt_sb[1], bk_sb[1]) if kt < 4
                                       else (wqs, qt[1], bq_sb[1]))
                    qb = kt % 4
                    qsl = slice(qb * 512, (qb + 1) * 512)
                    pp = sc_ps.tile([128, 512], F32, tag="sc", name="sc")
                    for et in range(ET):
                        nc.tensor.matmul(pp[:], wsrc[et][:, 128:256], xts[et][:, qsl],
                                         start=(et == 0), stop=(et == ET - 1))
                    nc.vector.tensor_scalar_add(dst[:, qsl], pp[:], bias[:])

                def attn_block(qh, t, weave):
                    hA, hB = 2 * t, 2 * t + 1
                    pslA, pslB = slice(0, 64), slice(64, 128)
                    pvpA = pv_ps.tile([HD + 1, 1024], F32, tag="pvA", name="pvA")
                    pvpB = pv_ps.tile([HD + 1, 1024], F32, tag="pvB", name="pvB")
                    for kt in range(ST):
                        if weave is not None:
                            weave(kt)
                        ksl = slice(kt * 128, (kt + 1) * 128)
                        scA = sc_ps.tile([128, 1024], F32, tag="sc", name="sc")
                        scB = sc_ps.tile([128, 1024], F32, tag="sc", name="sc")
                        for q2 in range(2):
                            qsl = slice(qh * 1024 + q2 * 512,
                                        qh * 1024 + (q2 + 1) * 512)
                            osl = slice(q2 * 512, (q2 + 1) * 512)
                            # adjacent row-group matmuls run concurrently
                            nc.tensor.matmul(scA[:, osl], kt_sb[t][pslA, ksl],
                                             qt[t][pslA, qsl], start=True, stop=True)
                            nc.tensor.matmul(scB[:, osl], kt_sb[t][pslB, ksl],
                                             qt[t][pslB, qsl], start=True, stop=True)
                        eA = e_pool.tile([128, 1024], MD, tag="e", name="e")
                        nc.scalar.activation(eA[:], scA[:], AF.Exp, scale=0.125)
                        eB = e_pool.tile([128, 1024], MD, tag="e", name="e")
                        nc.scalar.activation(eB[:], scB[:], AF.Exp, scale=0.125)
                        for q2 in range(2):
                            osl = slice(q2 * 512, (q2 + 1) * 512)
                            nc.tensor.matmul(pvpA[:, osl], v_sb[kt][:, hA, :],
                                             eA[:, osl],
                                             start=(kt == 0), stop=(kt == ST - 1))
                        for q2 in range(2):
                            osl = slice(q2 * 512, (q2 + 1) * 512)
                            nc.tensor.matmul(pvpB[:, osl], v_sb[kt][:, hB, :],
                                             eB[:, osl],
                                             start=(kt == 0), stop=(kt == ST - 1))
                    # normalize: U^T[0:64,:] * broadcast(1/denom row)
                    ssl = slice(qh * 1024, (qh + 1) * 1024)
                    for pvp, psl in ((pvpA, pslA), (pvpB, pslB)):
                        rc = rc_pool.tile([1, 1024], F32, tag="rc", name="rc")
                        nc.vector.reciprocal(rc[:], pvp[HD:HD + 1, :])
                        rcb = rcb_pool.tile([HD, 1024], F32, tag="rcb", name="rcb")
                        nc.gpsimd.partition_broadcast(rcb[:], rc[:], channels=HD)
                        nc.vector.tensor_tensor(ut[t][psl, ssl], pvp[0:HD, :],
                                                rcb[:], mybir.AluOpType.mult)

                attn_block(0, 0, weave_v)
                attn_block(1, 0, weave_kq1)
                attn_block(0, 1, None)
                attn_block(1, 1, None)

            # ---- stage C: out-projection ----------------------------------
            with tc.tile_pool(name="o_sb", bufs=3) as o_pool, \
                 tc.tile_pool(name="op_ps", bufs=2, space="PSUM") as op_ps:
                for st in range(ST):
                    ssl = slice(st * 128, (st + 1) * 128)
                    o_sb = o_pool.tile([128, E], mybir.dt.bfloat16, tag="o", name="o")
                    for ob in range(2):
                        osl = slice(ob * 512, (ob + 1) * 512)
                        op = op_ps.tile([128, 512], F32, tag="op", name="op")
                        for t in range(2):
                            nc.tensor.matmul(op[:], ut[t][:, ssl],
                                             wo_sb[t][:, osl],
                                             start=(t == 0), stop=(t == 1))
                        nc.vector.tensor_copy(o_sb[:, osl], op[:])
                    nc.sync.dma_start(out_d[ssl, :], o_sb[:])

    nc.compile()
    return nc


def _get_graph():
    if "nc" not in _CACHED:
        _CACHED["nc"] = build_core_graph()
    return _CACHED["nc"]


def _np_mode_dtype():
    if MODE == "bf16":
        import ml_dtypes
        return ml_dtypes.bfloat16
    return np.float32


def kernel(x, W_qkv, b_qkv, W_out, b_out):
    x = np.asarray(x, dtype=np.float32)
    W_qkv = np.asarray(W_qkv, dtype=np.float32)
    b_qkv = np.asarray(b_qkv, dtype=np.float32)
    W_out = np.asarray(W_out, dtype=np.float32)
    b_out = np.asarray(b_out, dtype=np.float32)

    nc = _get_graph()
    md = _np_mode_dtype()

    Wq, Wk, Wv = W_qkv[:, 0:E], W_qkv[:, E:2 * E], W_qkv[:, 2 * E:3 * E]
    bq, bk, bv = b_qkv[0:E], b_qkv[E:2 * E], b_qkv[2 * E:3 * E]

    in_maps = []
    for c in range(8):
        b, hg = c // 4, c % 4
        cols = slice(DQ * hg, DQ * hg + DQ)
        in_maps.append({
            "xt": np.ascontiguousarray(x[b].T).astype(md),
            "wq": np.ascontiguousarray(Wq[:, cols]).astype(md),
            "wk": np.ascontiguousarray(Wk[:, cols]).astype(md),
            "wv": np.ascontiguousarray(Wv[:, cols]).astype(md),
            "bq": np.ascontiguousarray(bq[cols].reshape(DQ, 1)),
            "bk": np.ascontiguousarray(bk[cols].reshape(DQ, 1)),
            "wo": np.ascontiguousarray(W_out[cols, :]).astype(md),
        })

    res = bass_utils.run_bass_kernel_spmd(nc, in_maps, core_ids=list(range(8)))
    _CACHED["last_results"] = res

    b_eff = (b_out.astype(np.float64) +
             bv.astype(np.float64) @ W_out.astype(np.float64))
    out = np.empty((2, S, E), np.float32)
    for b in range(2):
        acc = np.zeros((S, E), np.float64)
        for hg in range(4):
            acc += res.results[4 * b + hg]["out"].astype(np.float64)
        out[b] = (acc + b_eff).astype(np.float32)
    return out


# revision 10
# speedup vs baseline: 1.1168x; 1.1038x over previous
"""Distributed Trainium2 kernel for a 16-head self-attention block.

Model (reference):
    qkv = x @ W_qkv + b_qkv ; q,k,v per 16 heads (head_dim 64)
    attn = softmax(q k^T / 8) ; out = (attn @ v heads concat) @ W_out + b_out
Shapes: x [2, 2048, 1024], W_qkv [1024, 3072], W_out [1024, 1024].

Sharding (8 NeuronCores): 2 batch groups x 4 cores; each core owns one batch
element and 4 of the 16 heads (Megatron-style column-parallel QKV + row-
parallel out-proj). Each core computes its partial out-projection
[2048, 1024]; the host sums the 4 partials per batch (the Megatron
all-reduce, performed at unshard time) and adds the output bias.

Numerical notes:
  * softmax runs without max-subtraction: scores/8 here are bounded ~|2.5|.
  * the V bias folds out of attention exactly (softmax rows sum to 1), so
    bv@W_out is added to the host-side output bias instead of on device.

Device dataflow per core (no transposes anywhere):
  x^T [1024, 2048] (host pre-transposed) ->
  Q^T,K^T [256, 2048] = Wq^T x^T (+bias on eviction);  V [2048, 4, 65]
    (per-head 64 dims + a ones column, which makes the PV matmul emit the
    softmax denominator as row 64 of U^T)
  per head h, per 1024-query half: for each 128-key tile:
    scores^T [128 k, 1024 q] in a 2-bank PSUM tile (2 matmuls, shared
    stationary), exp via one 1024-wide ScalarE activation -> E [128, 1024]
    U^T [65, 512] += V[kt]^T E  (V stationary 65 cols, E moving 512)
  normalization: rc[1,512] = 1/denom row (DVE), GpSimd partition-broadcast
    to [64,512], one DVE multiply evicts U^T/denom into ut (bf16).
  out partial [2048, 1024] = (U^T)^T @ Wo_rows, emitted per query half so
  it overlaps the other half's attention.
"""

import contextlib
import os

import numpy as np

import concourse.bacc as bacc
import concourse.mybir as mybir
import concourse.tile as tile
from concourse import bass_utils

F32 = mybir.dt.float32
AF = mybir.ActivationFunctionType

S = 2048          # sequence length (one batch element per core)
E = 1024          # embedding dim
HD = 64           # head dim
NH = 4            # heads per core
DQ = NH * HD      # per-core q/k/v width (256)
ET = E // 128     # embedding tiles (8)
ST = S // 128     # sequence tiles (16)
QB = S // 512     # 512-wide query blocks (4)

MODE = os.environ.get("ATTN_MM_DTYPE", "bf16")  # bf16 | f32r | f32

_CACHED = {}


def build_core_graph(mode=MODE):
    MD = {"f32r": mybir.dt.float32r, "f32": F32, "bf16": mybir.dt.bfloat16}[mode]

    nc = bacc.Bacc("TRN2", target_bir_lowering=False, debug=False, num_devices=8)

    xt_d = nc.dram_tensor("xt", [E, S], MD, kind="ExternalInput")
    wq_d = nc.dram_tensor("wq", [E, DQ], MD, kind="ExternalInput")
    wk_d = nc.dram_tensor("wk", [E, DQ], MD, kind="ExternalInput")
    wv_d = nc.dram_tensor("wv", [E, DQ], MD, kind="ExternalInput")
    bq_d = nc.dram_tensor("bq", [DQ, 1], F32, kind="ExternalInput")
    bk_d = nc.dram_tensor("bk", [DQ, 1], F32, kind="ExternalInput")
    wo_d = nc.dram_tensor("wo", [DQ, E], MD, kind="ExternalInput")
    out_d = nc.dram_tensor("out", [S, E], mybir.dt.bfloat16, kind="ExternalOutput")

    with tile.TileContext(nc) as tc:
        with contextlib.ExitStack() as ctx:
            # ---- persistent SBUF pools ------------------------------------
            pers = ctx.enter_context(tc.tile_pool(name="pers", bufs=1))

            def ptile(shape, dtype, nm):
                return pers.tile(shape, dtype, tag=nm, name=nm)

            qt = [ptile([128, S], MD, f"qt{t}") for t in range(2)]
            ones_c = ptile([128, NH], F32, "ones_c")
            kt_sb = [ptile([128, S], MD, f"kt{t}") for t in range(2)]
            v_sb = [ptile([128, NH, HD + 1], MD, f"v{st}") for st in range(ST)]
            ut = [ptile([128, S], MD, f"ut{t}") for t in range(2)]
            wo_sb = [ptile([128, E], MD, f"wo{t}") for t in range(2)]
            bq_sb = [ptile([128, 1], F32, f"bq{t}") for t in range(2)]
            bk_sb = [ptile([128, 1], F32, f"bk{t}") for t in range(2)]

            nc.vector.memset(ones_c[:], 1.0)
            for t in range(2):
                nc.sync.dma_start(wo_sb[t][:], wo_d[t * 128:(t + 1) * 128, :])
                nc.sync.dma_start(bq_sb[t][:], bq_d[t * 128:(t + 1) * 128, :])
                nc.sync.dma_start(bk_sb[t][:], bk_d[t * 128:(t + 1) * 128, :])

            # x^T and weight tiles live until the woven K1/Q1 block is done.
            apool = ctx.enter_context(tc.tile_pool(name="ab_sbuf", bufs=1))

            def atile(shape, nm):
                return apool.tile(shape, MD, tag=nm, name=nm)

            xts = [atile([128, S], f"xt{et}") for et in range(ET)]
            wqs = [atile([128, DQ], f"wq{et}") for et in range(ET)]
            wks = [atile([128, DQ], f"wk{et}") for et in range(ET)]
            wvs = [atile([128, DQ], f"wv{et}") for et in range(ET)]
            for et in range(ET):
                sl = slice(et * 128, (et + 1) * 128)
                nc.sync.dma_start(xts[et][:], xt_d[sl, :])
                nc.sync.dma_start(wqs[et][:], wq_d[sl, :])
                nc.sync.dma_start(wks[et][:], wk_d[sl, :])
                nc.sync.dma_start(wvs[et][:], wv_d[sl, :])

            # ---- stage A-pre: warmup + K/Q for head-pair 0 ----------------
            with tc.tile_pool(name="a_ps", bufs=4, space="PSUM") as a_ps:
                # HAM warm-up: keep the PE busy during the input-DMA window
                # so the clock-gate releases (K=8/8) before the real matmul
                # stream begins. Garbage data, never read.
                warm_sb = apool.tile([128, 512], MD, tag="warm", name="warm")
                nc.vector.memset(warm_sb[:], 0.0)
                wps = a_ps.tile([128, 512], F32, tag="wps", name="wps")
                for _ in range(56):
                    nc.tensor.matmul(wps[:], warm_sb[:, 0:128], warm_sb[:],
                                     start=True, stop=True)

                tsl = slice(0, 128)
                for qb in range(QB):
                    qsl = slice(qb * 512, (qb + 1) * 512)
                    pk = a_ps.tile([128, 512], F32, tag="aps", name="aps")
                    for et in range(ET):
                        nc.tensor.matmul(pk[:], wks[et][:, tsl], xts[et][:, qsl],
                                         start=(et == 0), stop=(et == ET - 1))
                    nc.vector.tensor_scalar_add(kt_sb[0][:, qsl], pk[:], bk_sb[0][:])
                    pq = a_ps.tile([128, 512], F32, tag="aps", name="aps")
                    for et in range(ET):
                        nc.tensor.matmul(pq[:], wqs[et][:, tsl], xts[et][:, qsl],
                                         start=(et == 0), stop=(et == ET - 1))
                    nc.vector.tensor_scalar_add(qt[0][:, qsl], pq[:], bq_sb[0][:])

            # ---- stage B: attention, two heads of a pair interleaved ------
            # PSUM budget (8 banks): sc ring 2 bufs x 2 banks = 4,
            # pvA + pvB [65,1024] 1 buf x 2 banks each = 4.
            # The V projection weaves into block 1's sc ring; K1/Q1 weave
            # into block 2's, so ScalarE starts exp'ing ~40us earlier.
            with tc.tile_pool(name="e_sb", bufs=4) as e_pool, \
                 tc.tile_pool(name="rc_sb", bufs=2) as rc_pool, \
                 tc.tile_pool(name="rcb_sb", bufs=2) as rcb_pool, \
                 tc.tile_pool(name="o_sb", bufs=3) as o_pool, \
                 tc.tile_pool(name="sc_ps", bufs=2, space="PSUM") as sc_ps, \
                 tc.tile_pool(name="pv_ps", bufs=1, space="PSUM") as pv_ps:

                def out_proj(st):
                    ssl = slice(st * 128, (st + 1) * 128)
                    op = sc_ps.tile([128, E], F32, tag="sc", name="sc")
                    for ob in range(2):
                        osl = slice(ob * 512, (ob + 1) * 512)
                        for t2 in range(2):
                            nc.tensor.matmul(op[:, osl], ut[t2][:, ssl],
                                             wo_sb[t2][:, osl],
                                             start=(t2 == 0), stop=(t2 == 1))
                    o_sb = o_pool.tile([128, E], mybir.dt.bfloat16, tag="o", name="o")
                    nc.vector.tensor_copy(o_sb[:], op[:])
                    nc.sync.dma_start(out_d[ssl, :], o_sb[:])

                def weave_v(kt):
                    ssl = slice(kt * 128, (kt + 1) * 128)
                    pv = sc_ps.tile([128, DQ], F32, tag="sc", name="sc")
                    for et in range(ET):
                        nc.tensor.matmul(pv[:], xts[et][:, ssl], wvs[et][:],
                                         start=(et == 0), stop=(et == ET - 1))
                    for hh in range(NH):
                        nc.vector.tensor_copy(v_sb[kt][:, hh, 0:HD],
                                              pv[:, hh * HD:(hh + 1) * HD])
                    nc.vector.tensor_copy(v_sb[kt][:, :, HD:HD + 1],
                                          ones_c[:].rearrange("p (h d) -> p h d", h=NH))

                def weave_kq1(kt):
                    if kt >= 8:
                        return
                    wsrc, dst, bias = ((wks, kt_sb[1], bk_sb[1]) if kt < 4
                                       else (wqs, qt[1], bq_sb[1]))
                    qb = kt % 4
                    qsl = slice(qb * 512, (qb + 1) * 512)
                    pp = sc_ps.tile([128, 512], F32, tag="sc", name="sc")
                    for et in range(ET):
                        nc.tensor.matmul(pp[:], wsrc[et][:, 128:256], xts[et][:, qsl],
                                         start=(et == 0), stop=(et == ET - 1))
                    nc.vector.tensor_scalar_add(dst[:, qsl], pp[:], bias[:])

                def attn_block(qh, t, weave, fill=False):
                    hA, hB = 2 * t, 2 * t + 1
                    pslA, pslB = slice(0, 64), slice(64, 128)
                    pvpA = pv_ps.tile([HD + 1, 1024], F32, tag="pvA", name="pvA")
                    pvpB = pv_ps.tile([HD + 1, 1024], F32, tag="pvB", name="pvB")
                    for kt in range(ST):
                        if weave is not None:
                            weave(kt)
                        ksl = slice(kt * 128, (kt + 1) * 128)
                        scA = sc_ps.tile([128, 1024], F32, tag="sc", name="sc")
                        scB = sc_ps.tile([128, 1024], F32, tag="sc", name="sc")
                        if fill:
                            # full-array filler into the about-to-be-overwritten
                            # bank: keeps the PE activity dense so the HAM
                            # clock-gate stays at full rate. Output is garbage
                            # and immediately overwritten (start=True below).
                            nc.tensor.matmul(scA[:, 0:512], warm_sb[:, 0:128],
                                             warm_sb[:], start=True, stop=True,
                                             skip_group_check=True)
                        for q2 in range(2):
                            qsl = slice(qh * 1024 + q2 * 512,
                                        qh * 1024 + (q2 + 1) * 512)
                            osl = slice(q2 * 512, (q2 + 1) * 512)
                            # adjacent row-group matmuls run concurrently
                            nc.tensor.matmul(scA[:, osl], kt_sb[t][pslA, ksl],
                                             qt[t][pslA, qsl], start=True, stop=True)
                            nc.tensor.matmul(scB[:, osl], kt_sb[t][pslB, ksl],
                                             qt[t][pslB, qsl], start=True, stop=True)
                        eA = e_pool.tile([128, 1024], MD, tag="e", name="e")
                        nc.scalar.activation(eA[:], scA[:], AF.Exp, scale=0.125)
                        eB = e_pool.tile([128, 1024], MD, tag="e", name="e")
                        nc.scalar.activation(eB[:], scB[:], AF.Exp, scale=0.125)
                        for q2 in range(2):
                            osl = slice(q2 * 512, (q2 + 1) * 512)
                            nc.tensor.matmul(pvpA[:, osl], v_sb[kt][:, hA, :],
                                             eA[:, osl],
                                             start=(kt == 0), stop=(kt == ST - 1))
                        for q2 in range(2):
                            osl = slice(q2 * 512, (q2 + 1) * 512)
                            nc.tensor.matmul(pvpB[:, osl], v_sb[kt][:, hB, :],
                                             eB[:, osl],
                                             start=(kt == 0), stop=(kt == ST - 1))
                    # normalize: U^T[0:64,:] * broadcast(1/denom row)
                    ssl = slice(qh * 1024, (qh + 1) * 1024)
                    for pvp, psl in ((pvpA, pslA), (pvpB, pslB)):
                        rc = rc_pool.tile([1, 1024], F32, tag="rc", name="rc")
                        nc.vector.reciprocal(rc[:], pvp[HD:HD + 1, :])
                        rcb = rcb_pool.tile([HD, 1024], F32, tag="rcb", name="rcb")
                        nc.gpsimd.partition_broadcast(rcb[:], rc[:], channels=HD)
                        nc.vector.tensor_tensor(ut[t][psl, ssl], pvp[0:HD, :],
                                                rcb[:], mybir.AluOpType.mult)

                attn_block(0, 0, weave_v)
                attn_block(1, 0, weave_kq1)
                attn_block(0, 1, None, fill=True)
                # block 4 weaves the first half of the out-projection
                # (query half 0's ut is complete after block 3).
                attn_block(1, 1, lambda kt: out_proj(kt) if kt < 8 else None,
                           fill=True)
                # remaining out-projection rides the same sc ring
                for st in range(8, ST):
                    out_proj(st)

    nc.compile()
    return nc


def _get_graph():
    if "nc" not in _CACHED:
        _CACHED["nc"] = build_core_graph()
    return _CACHED["nc"]


def _np_mode_dtype():
    if MODE == "bf16":
        import ml_dtypes
        return ml_dtypes.bfloat16
    return np.float32


def kernel(x, W_qkv, b_qkv, W_out, b_out):
    x = np.asarray(x, dtype=np.float32)
    W_qkv = np.asarray(W_qkv, dtype=np.float32)
    b_qkv = np.asarray(b_qkv, dtype=np.float32)
    W_out = np.asarray(W_out, dtype=np.float32)
    b_out = np.asarray(b_out, dtype=np.float32)

    nc = _get_graph()
    md = _np_mode_dtype()

    Wq, Wk, Wv = W_qkv[:, 0:E], W_qkv[:, E:2 * E], W_qkv[:, 2 * E:3 * E]
    bq, bk, bv = b_qkv[0:E], b_qkv[E:2 * E], b_qkv[2 * E:3 * E]

    in_maps = []
    for c in range(8):
        b, hg = c // 4, c % 4
        cols = slice(DQ * hg, DQ * hg + DQ)
        in_maps.append({
            "xt": np.ascontiguousarray(x[b].T).astype(md),
            "wq": np.ascontiguousarray(Wq[:, cols]).astype(md),
            "wk": np.ascontiguousarray(Wk[:, cols]).astype(md),
            "wv": np.ascontiguousarray(Wv[:, cols]).astype(md),
            "bq": np.ascontiguousarray(bq[cols].reshape(DQ, 1)),
            "bk": np.ascontiguousarray(bk[cols].reshape(DQ, 1)),
            "wo": np.ascontiguousarray(W_out[cols, :]).astype(md),
        })

    res = bass_utils.run_bass_kernel_spmd(nc, in_maps, core_ids=list(range(8)))
    _CACHED["last_results"] = res

    b_eff = (b_out.astype(np.float64) +
             bv.astype(np.float64) @ W_out.astype(np.float64))
    out = np.empty((2, S, E), np.float32)
    for b in range(2):
        acc = np.zeros((S, E), np.float64)
        for hg in range(4):
            acc += res.results[4 * b + hg]["out"].astype(np.float64)
        out[b] = (acc + b_eff).astype(np.float32)
    return out
